# revision 22
# baseline (speedup 1.0000x reference)
"""Trainium2 Bass kernel: BERT-style self-attention with granularity-gated
sparse penalties (softmax(QK^T/sqrt(d) + log(penalties)) @ V).

Math restructure (exact up to ~1e-24 relative):
  softmax(S + log(max(pen, 1e-32))) == pen * exp(S) / sum_j(pen * exp(S))
  - no log needed, no max-subtraction (S bounded ~|25|, exp can't overflow)
  - masked entries (scope clipped at 0 instead of 1e-32) contribute 0

Layout: S^T tiles [128 keys x 512 queries] so the softmax reduction over keys
is a matmul contraction: l = ones-row folded into V_aug's 65th column.

Sharding: core c in 0..7 -> batch b=c//4, query slab q0=(c%4)*512, all 16
heads, all 2048 keys. Penalties [2048k x 512q] computed once per core in SBUF
(bf16), reused by all 16 heads.

Precision: fp16 for hidden/W/Q/K (score path), bf16 for V/E/pen (exp values
exceed fp16 range), f32 PSUM accumulation everywhere, query-side gate g via
fp16 hi+lo compensated matmul (band boundary exact to ~0.02 tokens).
"""

import math

import ml_dtypes
import numpy as np

import concourse.bass as bass
import concourse.tile as tile
from concourse import bacc, mybir
from concourse.bass import AP
from concourse.bass_utils import run_bass_kernel_spmd
from concourse.masks import make_identity

F32 = mybir.dt.float32
BF16 = mybir.dt.bfloat16
FP16 = mybir.dt.float16
AF = mybir.ActivationFunctionType
OP = mybir.AluOpType

B, S, H = 2, 2048, 1024
NH, HD = 16, 64
NC = 8
SLAB = S // 4          # 512 queries per core
KT = S // 128          # 16 key tiles
HT = 9                 # ceil(1025/128) contraction tiles (1024 + bias row)
LN_BASE = float(np.log(np.float32(S - 2)))  # ln(2046)
VW = HD + 1            # 65: V columns + ones column per (kt, head)


def build_nc():
    nc = bacc.Bacc("TRN2", target_bir_lowering=False, debug=False)

    hT = nc.dram_tensor("hT", [H, S], FP16, kind="ExternalInput").ap()
    hTq = nc.dram_tensor("hTq", [H, SLAB], FP16, kind="ExternalInput").ap()
    hTq_lo = nc.dram_tensor("hTq_lo", [H, SLAB], FP16, kind="ExternalInput").ap()
    Wq = nc.dram_tensor("Wq", [8, H + 1, 128], FP16, kind="ExternalInput").ap()
    Wk = nc.dram_tensor("Wk", [8, H + 1, 128], FP16, kind="ExternalInput").ap()
    Wv = nc.dram_tensor("Wv", [2, H + 1, 512], FP16, kind="ExternalInput").ap()
    Wg = nc.dram_tensor("Wg", [H, 1], FP16, kind="ExternalInput").ap()
    Wg_lo = nc.dram_tensor("Wg_lo", [H, 1], FP16, kind="ExternalInput").ap()
    bqv = nc.dram_tensor("bqv", [H], F32, kind="ExternalInput").ap()
    bkv = nc.dram_tensor("bkv", [H], F32, kind="ExternalInput").ap()
    bvp = nc.dram_tensor("bvp", [VW, NH], F32, kind="ExternalInput").ap()
    bgv = nc.dram_tensor("bgv", [1], F32, kind="ExternalInput").ap()
    idx = nc.dram_tensor("idx", [S], F32, kind="ExternalInput").ap()
    idxq = nc.dram_tensor("idxq", [SLAB], F32, kind="ExternalInput").ap()
    out = nc.dram_tensor("out", [SLAB, H], F32, kind="ExternalOutput").ap()

    with tile.TileContext(nc) as tc:
        _body(tc, nc, hT, hTq, hTq_lo, Wq, Wk, Wv, Wg, Wg_lo, bqv, bkv, bvp, bgv, idx, idxq, out)
    nc.compile()
    return nc


def _bcast(ap_1d, n_part, n_free):
    """[n] dram AP -> [n_part, n_free] partition-broadcast AP."""
    return AP(tensor=ap_1d.tensor, offset=ap_1d.offset, ap=[[0, n_part], [1, n_free]])


def _pm_view(ap_1d, n_free):
    """[128*n_free] dram AP <-> [128, n_free] partition-major SBUF tile:
    element (p, f) maps to dram[p + 128*f]."""
    return AP(tensor=ap_1d.tensor, offset=ap_1d.offset, ap=[[1, 128], [128, n_free]])


def _body(tc, nc, hT, hTq, hTq_lo, Wq, Wk, Wv, Wg, Wg_lo, bqv, bkv, bvp, bgv, idx, idxq, out):
    import contextlib

    ctx = contextlib.ExitStack()
    with ctx:
        big = ctx.enter_context(tc.tile_pool(name="big", bufs=1))
        consts = ctx.enter_context(tc.tile_pool(name="consts", bufs=1))
        dram = ctx.enter_context(tc.tile_pool(name="dram", bufs=1, space="DRAM"))
        wk_pool = ctx.enter_context(tc.tile_pool(name="wk", bufs=2))
        wv_pool = ctx.enter_context(tc.tile_pool(name="wv", bufs=1))
        # proj psum pool — stays open through attention (reused for transposes)
        psp = ctx.enter_context(tc.tile_pool(name="psp", bufs=2, space="PSUM"))

        # --- resident SBUF tensors ---
        hT_sb = big.tile([128, 8 * S], FP16)           # h-tile major
        hTq_sb = big.tile([128, 8 * SLAB], FP16)
        hTq_lo_sb = big.tile([128, 8 * SLAB], FP16)
        qT_sb = big.tile([128, 8 * SLAB], FP16)        # d-tile major
        ktT_sb = big.tile([128, 8 * S], FP16)          # d-tile major
        v_sb = big.tile([128, KT * NH * VW], BF16)     # kt major, per-head 65
        pen_sb = big.tile([128, KT * SLAB], BF16)      # kt major

        ident = consts.tile([128, 128], F32)
        make_identity(nc, ident)

        # load hidden transposed and index vectors
        for ht in range(8):
            nc.sync.dma_start(
                hT_sb[:, ht * S : ht * S + S], hT[ht * 128 : ht * 128 + 128, :]
            )
            nc.sync.dma_start(
                hTq_sb[:, ht * SLAB : ht * SLAB + SLAB],
                hTq[ht * 128 : ht * 128 + 128, :],
            )
            nc.sync.dma_start(
                hTq_lo_sb[:, ht * SLAB : ht * SLAB + SLAB],
                hTq_lo[ht * 128 : ht * 128 + 128, :],
            )
        wg_sb = consts.tile([128, 8], FP16)
        wg_lo_sb = consts.tile([128, 8], FP16)
        for ht in range(8):
            nc.sync.dma_start(
                wg_sb[:, ht : ht + 1], Wg[ht * 128 : ht * 128 + 128, :]
            )
            nc.sync.dma_start(
                wg_lo_sb[:, ht : ht + 1], Wg_lo[ht * 128 : ht * 128 + 128, :]
            )

        idx_pm = consts.tile([128, KT], F32)
        nc.sync.dma_start(idx_pm[:, :], _pm_view(idx, KT))
        nidx_pm = consts.tile([128, KT], F32)
        nc.vector.tensor_scalar(nidx_pm[:, :], idx_pm[:, :], -1.0, None, OP.mult)
        idxq_pm = consts.tile([128, SLAB // 128], F32)
        nc.sync.dma_start(idxq_pm[:, :], _pm_view(idxq, SLAB // 128))

        # ---- granularity gate g (row layout: [1, N] matmuls, N=512) ----
        nq = SLAB // 128
        bg_sb = consts.tile([1, 1], F32)
        nc.sync.dma_start(bg_sb[:, :], bgv[None, :])
        bk_sb = consts.tile([128, 8], F32)
        nc.sync.dma_start(bk_sb[:, :], _pm_view(bkv, 8))
        bq_sb = consts.tile([128, 8], F32)
        nc.sync.dma_start(bq_sb[:, :], _pm_view(bqv, 8))
        bvp_sb = consts.tile([VW, NH], F32)
        nc.sync.dma_start(bvp_sb[:, :], bvp[:, :])

        grow_ctx = tc.tile_pool(name="grow", bufs=1)
        grow = grow_ctx.__enter__()
        z_row = grow.tile([1, S], F32)
        zq_row = grow.tile([1, SLAB], F32)
        with tc.tile_pool(name="psg", bufs=1, space="PSUM") as psg:
            g_ps = psg.tile([1, S], F32)
            for ht in range(8):
                for tb in range(4):
                    nc.tensor.matmul(
                        g_ps[:, tb * 512 : (tb + 1) * 512],
                        wg_sb[:, ht : ht + 1],
                        hT_sb[:, ht * S + tb * 512 : ht * S + (tb + 1) * 512],
                        start=(ht == 0),
                        stop=(ht == 7),
                    )
            nc.scalar.activation(z_row[:, :], g_ps[:, :], AF.Sigmoid, bias=bg_sb[:, :])

            gq_ps = psg.tile([1, SLAB], F32)
            first = True
            for ht in range(8):
                for wg_t, h_t in (
                    (wg_sb, hTq_sb),
                    (wg_lo_sb, hTq_sb),
                    (wg_sb, hTq_lo_sb),
                ):
                    nc.tensor.matmul(
                        gq_ps[:, :],
                        wg_t[:, ht : ht + 1],
                        h_t[:, ht * SLAB : (ht + 1) * SLAB],
                        start=first,
                        stop=(ht == 7 and h_t is hTq_lo_sb),
                    )
                    first = False
            nc.scalar.activation(
                zq_row[:, :], gq_ps[:, :], AF.Sigmoid, bias=bg_sb[:, :]
            )

        # key-side per-partition scalars via DRAM round-trip
        zrow_d = dram.tile([S], F32)
        nc.sync.dma_start(zrow_d[:], z_row[:, :])
        z_pm = consts.tile([128, KT], F32)
        nc.sync.dma_start(z_pm[:, :], _pm_view(zrow_d, KT))
        negz_pm = consts.tile([128, KT], F32)
        nc.vector.tensor_scalar(negz_pm[:, :], z_pm[:, :], -1.0, None, OP.mult)
        a_pm = consts.tile([128, KT], F32)
        nc.vector.tensor_scalar(a_pm[:, :], z_pm[:, :], -1.0, 1.0, OP.mult, OP.add)

        # query-side derived vectors in row layout
        idxq_row = grow.tile([1, SLAB], F32)
        nc.sync.dma_start(idxq_row[:, :], idxq[None, :])
        lnb = grow.tile([1, 1], F32)
        nc.vector.memset(lnb[:, :], LN_BASE)
        wq_row = grow.tile([1, SLAB], F32)
        nc.scalar.activation(
            wq_row[:, :], zq_row[:, :], AF.Exp, bias=lnb[:, :], scale=-LN_BASE
        )
        w2_row = grow.tile([1, SLAB], F32)
        nc.vector.tensor_scalar(w2_row[:, :], wq_row[:, :], 2.0, None, OP.add)
        al_row = grow.tile([1, SLAB], F32)
        nc.vector.tensor_sub(al_row[:, :], w2_row[:, :], idxq_row[:, :])
        be_row = grow.tile([1, SLAB], F32)
        nc.vector.tensor_add(be_row[:, :], w2_row[:, :], idxq_row[:, :])
        zq_bf = grow.tile([1, SLAB], BF16)
        nc.vector.tensor_scalar(zq_bf[:, :], zq_row[:, :], 0.0, None, OP.add)
        nzq_bf = grow.tile([1, SLAB], BF16)
        nc.vector.tensor_scalar(nzq_bf[:, :], zq_row[:, :], -1.0, None, OP.mult)
        aq_bf = grow.tile([1, SLAB], BF16)
        nc.vector.tensor_scalar(aq_bf[:, :], zq_row[:, :], -1.0, 1.0, OP.mult, OP.add)

        # scatter to per-core DRAM scratch, then broadcast-read
        zq_d = dram.tile([SLAB], BF16)
        nzq_d = dram.tile([SLAB], BF16)
        aq_d = dram.tile([SLAB], BF16)
        al_d = dram.tile([SLAB], F32)
        be_d = dram.tile([SLAB], F32)
        for src_t, dst in (
            (zq_bf, zq_d),
            (nzq_bf, nzq_d),
            (aq_bf, aq_d),
            (al_row, al_d),
            (be_row, be_d),
        ):
            nc.sync.dma_start(dst[None, :], src_t[:, :])
        BZ = consts.tile([128, SLAB], BF16)
        BnegZ = consts.tile([128, SLAB], BF16)
        BA = consts.tile([128, SLAB], BF16)
        Balpha = consts.tile([128, SLAB], F32)
        Bbeta = consts.tile([128, SLAB], F32)
        for src_t, dst in (
            (zq_d, BZ),
            (nzq_d, BnegZ),
            (aq_d, BA),
            (al_d, Balpha),
            (be_d, Bbeta),
        ):
            nc.sync.dma_start(dst[:, :], _bcast(src_t, 128, SLAB))
        grow_ctx.__exit__(None, None, None)
        penw = ctx.enter_context(tc.tile_pool(name="penw", bufs=2))
        epool = ctx.enter_context(tc.tile_pool(name="ep", bufs=8))
        cpool = ctx.enter_context(tc.tile_pool(name="cp", bufs=2))
        opool = ctx.enter_context(tc.tile_pool(name="op", bufs=2))

        # ---- V projection: tv-major per d-half so PV can chase it ----
        ones_view = v_sb[:, :].rearrange("p (k c) -> p k c", c=VW)[:, :, HD : HD + 1]
        nc.gpsimd.memset(ones_view, 1.0)

        def v_proj_pass(vd):
            wts = [
                wv_pool.tile([128, 512], FP16, tag=f"wv{ht}", name=f"wv{ht}")
                for ht in range(8)
            ]
            for ht in range(8):
                nc.sync.dma_start(
                    wts[ht][:, :], Wv[vd, ht * 128 : ht * 128 + 128, :]
                )
            for tv in range(KT):
                ps = psp.tile([128, 512], F32, tag="ps", name="psv")
                for ht in range(8):
                    nc.tensor.matmul(
                        ps[:, :],
                        hT_sb[:, ht * S + tv * 128 : ht * S + tv * 128 + 128],
                        wts[ht][:, :],
                        start=(ht == 0),
                        stop=(ht == 7),
                    )
                base = tv * NH * VW + vd * 8 * VW
                dst = v_sb[:, base : base + 8 * VW].rearrange(
                    "p (h c) -> p h c", c=VW
                )[:, :, 0:HD]
                src = ps[:, :].rearrange("p (h c) -> p h c", c=HD)
                nc.scalar.copy(dst, src)

        # ---- K^T and Q^T per d-tile ----
        def _copy_ps(dst, ps, bias_ap, use_dve):
            if use_dve:
                # DVE: add per-partition bias then cast
                nc.vector.tensor_scalar(dst, ps, bias_ap, None, OP.add)
            else:
                nc.scalar.activation(dst, ps, AF.Identity, bias=bias_ap)

        def kq_proj_units(dt):
            wkts = [
                wk_pool.tile([128, 128], FP16, tag=f"wk{ht}", name=f"wk{ht}")
                for ht in range(8)
            ]
            wqts = [
                wk_pool.tile([128, 128], FP16, tag=f"wq{ht}", name=f"wq{ht}")
                for ht in range(8)
            ]
            for ht in range(8):
                nc.sync.dma_start(
                    wkts[ht][:, :], Wk[dt, ht * 128 : ht * 128 + 128, :]
                )
                nc.sync.dma_start(
                    wqts[ht][:, :], Wq[dt, ht * 128 : ht * 128 + 128, :]
                )

            def k_unit(tt, use_dve):
                ps = psp.tile([128, 512], F32, tag="ps", name="psk")
                for ht in range(8):
                    nc.tensor.matmul(
                        ps[:, :],
                        wkts[ht][:, :],
                        hT_sb[:, ht * S + tt * 512 : ht * S + (tt + 1) * 512],
                        start=(ht == 0),
                        stop=(ht == 7),
                    )
                _copy_ps(
                    ktT_sb[:, dt * S + tt * 512 : dt * S + (tt + 1) * 512],
                    ps[:, :],
                    bk_sb[:, dt : dt + 1],
                    use_dve,
                )

            def q_unit(use_dve):
                ps = psp.tile([128, SLAB], F32, tag="ps", name="psq")
                for ht in range(8):
                    nc.tensor.matmul(
                        ps[:, :],
                        wqts[ht][:, :],
                        hTq_sb[:, ht * SLAB : (ht + 1) * SLAB],
                        start=(ht == 0),
                        stop=(ht == 7),
                    )
                _copy_ps(
                    qT_sb[:, dt * SLAB : (dt + 1) * SLAB],
                    ps[:, :],
                    bq_sb[:, dt : dt + 1],
                    use_dve,
                )

            units = [lambda tt=tt: k_unit(tt, True) for tt in range(4)]
            units.append(lambda: q_unit(False))
            return units

        for u in kq_proj_units(0):
            u()
        for u in kq_proj_units(1):
            u()

        # ---- penalties pen^T        # ---- penalties pen^T [128 keys x 512 queries] per key-tile ----
        # (program-order after proj so proj copies win engine priority; runs
        # on DVE concurrently with proj's PE work)
        for kt in range(KT):
            aj = a_pm[:, kt : kt + 1]
            nzj = negz_pm[:, kt : kt + 1]
            jp = idx_pm[:, kt : kt + 1]
            njp = nidx_pm[:, kt : kt + 1]
            r1 = penw.tile([128, SLAB], BF16, tag="r1")
            nc.vector.tensor_scalar(r1[:, :], BnegZ[:, :], aj, 0.0, OP.add, OP.max)
            r2 = penw.tile([128, SLAB], BF16, tag="r2")
            nc.vector.tensor_scalar(r2[:, :], BZ[:, :], nzj, 0.0, OP.add, OP.max)
            t = penw.tile([128, SLAB], BF16, tag="t")
            nc.vector.tensor_mul(t[:, :], BA[:, :], r1[:, :])
            # u = (r2 - 1) * z_i  (so res = t - u = a*r1 + z*(1-r2))
            u = penw.tile([128, SLAB], BF16, tag="u")
            nc.vector.scalar_tensor_tensor(
                u[:, :], r2[:, :], 1.0, BZ[:, :], OP.subtract, OP.mult
            )
            res = penw.tile([128, SLAB], BF16, tag="res")
            nc.vector.tensor_sub(res[:, :], t[:, :], u[:, :])
            # scope = clip(min(alpha+j, beta-j), 0, 1)
            s1 = penw.tile([128, SLAB], F32, tag="s1")
            nc.vector.tensor_scalar(s1[:, :], Balpha[:, :], jp, 1.0, OP.add, OP.min)
            sc = penw.tile([128, SLAB], F32, tag="sc")
            nc.vector.scalar_tensor_tensor(
                sc[:, :], Bbeta[:, :], njp, s1[:, :], OP.add, OP.min
            )
            scb = penw.tile([128, SLAB], BF16, tag="scb")
            nc.vector.tensor_scalar(scb[:, :], sc[:, :], 0.0, None, OP.max)
            nc.vector.tensor_mul(
                pen_sb[:, kt * SLAB : (kt + 1) * SLAB], res[:, :], scb[:, :]
            )

        # ---- V passes then attention (PE filler: V vd=1 + K/Q dt>=2) ----
        v_proj_pass(0)
        v_proj_pass(1)
        with (
            tc.tile_pool(name="pss", bufs=2, space="PSUM") as pss,
            tc.tile_pool(name="psv2", bufs=1, space="PSUM") as psv2,
        ):
            def emit_epilogue(pend):
                og, ctxTs = pend
                for h, ctxT in ctxTs:
                    for qt in range(4):
                        tp = psp.tile([128, VW], F32, tag="ps", name="tp")
                        nc.tensor.transpose(
                            tp[:, :],
                            ctxT[:, qt * 128 : (qt + 1) * 128],
                            ident[:VW, :VW],
                        )
                        rc = opool.tile([128, 1], F32, tag="rc")
                        nc.vector.reciprocal(rc[:, :], tp[:, HD : HD + 1])
                        nc.vector.tensor_scalar(
                            og[:, qt, (h % 2) * HD : (h % 2) * HD + HD],
                            tp[:, 0:HD],
                            rc[:, :],
                            None,
                            OP.mult,
                        )
                hh0 = ctxTs[0][0]
                for qt in range(4):
                    nc.sync.dma_start(
                        out[qt * 128 : (qt + 1) * 128, hh0 * HD : hh0 * HD + 128],
                        og[:, qt, :],
                    )

            pending = None
            for g in range(8):
                h0, h1 = 2 * g, 2 * g + 1
                units = kq_proj_units(g + 2) if g + 2 < 8 else []
                pv0 = psv2.tile([VW, 512], F32, tag="pv0")
                pv1 = psv2.tile([VW, 512], F32, tag="pv1")
                def pv_mms(kt, e):
                    nc.tensor.matmul(
                        pv0,
                        v_sb[:, kt * NH * VW + h0 * VW : kt * NH * VW + (h0 + 1) * VW],
                        e[:, 0:512],
                        start=(kt == 0),
                        stop=(kt == KT - 1),
                    )
                    nc.tensor.matmul(
                        pv1,
                        v_sb[:, kt * NH * VW + h1 * VW : kt * NH * VW + (h1 + 1) * VW],
                        e[:, 512:1024],
                        start=(kt == 0),
                        stop=(kt == KT - 1),
                    )

                e_q = []
                for kt in range(KT):
                    sp = pss.tile([128, 1024], F32, tag="sp")
                    nc.tensor.matmul(
                        sp[:, 0:512],
                        ktT_sb[0:64, g * S + kt * 128 : g * S + kt * 128 + 128],
                        qT_sb[0:64, g * SLAB : (g + 1) * SLAB],
                        start=True,
                        stop=True,
                        tile_position=(0, 0),
                    )
                    nc.tensor.matmul(
                        sp[:, 512:1024],
                        ktT_sb[64:128, g * S + kt * 128 : g * S + kt * 128 + 128],
                        qT_sb[64:128, g * SLAB : (g + 1) * SLAB],
                        start=True,
                        stop=True,
                        tile_position=(64, 0),
                    )
                    if kt == 1 and pending is not None:
                        emit_epilogue(pending)
                        pending = None
                    # PV for kt-2: its E is ready; keeps the in-order PE
                    # queue from head-of-line blocking on the exp/mul chain
                    if e_q and len(e_q) > 1:
                        pv_mms(*e_q.pop(0))
                    if kt % 3 == 2 and units:
                        units.pop(0)()
                    e = epool.tile([128, 1024], BF16, tag="e")
                    nc.scalar.activation(
                        e[:, :], sp[:, :], AF.Exp, scale=1.0 / math.sqrt(HD)
                    )
                    pen_slice = pen_sb[:, kt * SLAB : (kt + 1) * SLAB]
                    pen_b = AP(
                        tensor=pen_slice.tensor,
                        offset=pen_slice.offset,
                        ap=[pen_slice.ap[0], [0, 2], pen_slice.ap[1]],
                    )
                    e_view = e[:, :].rearrange("p (r n) -> p r n", r=2)
                    if g < 2 or kt % 3 == 2:
                        nc.gpsimd.tensor_mul(e_view, e_view, pen_b)
                    else:
                        nc.vector.tensor_mul(e_view, e_view, pen_b)
                    e_q.append((kt, e))
                for kt_e in e_q:
                    pv_mms(*kt_e)
                for u in units:
                    u()
                og = opool.tile([128, 4, 128], F32, tag="og")
                ctxTs = []
                for h, pv in ((h0, pv0), (h1, pv1)):
                    ctxT = cpool.tile([VW, 512], F32, tag="ctxT")
                    if h % 2 == 0:
                        nc.scalar.activation(
                            ctxT[:, :], pv[:, :], AF.Identity,
                            bias=bvp_sb[:, h : h + 1],
                        )
                    else:
                        nc.vector.tensor_scalar(
                            ctxT[:, :], pv[:, :], bvp_sb[:, h : h + 1], None, OP.add
                        )
                    ctxTs.append((h, ctxT))
                pending = (og, ctxTs)
            emit_epilogue(pending)


_NC_CACHE = None


def _get_nc():
    global _NC_CACHE
    if _NC_CACHE is None:
        _NC_CACHE = build_nc()
    return _NC_CACHE


def _prep_inputs(hidden_states, Wq, bq, Wk, bk, Wv, bv, Wg, bg):
    f16 = np.float16
    hidden_states = np.asarray(hidden_states, np.float32)

    def aug(W, b):
        W = np.asarray(W, np.float32)
        b = np.asarray(b, np.float32).reshape(1, -1)
        return np.vstack([W, b]).astype(np.float32)

    def tile_w(W, width):
        # [1024, H] -> [H//width, 1025, width] contiguous blocks (row 1024 pad)
        Wa = np.vstack([np.asarray(W, np.float32), np.zeros((1, H), np.float32)])
        n = H // width
        return np.ascontiguousarray(
            Wa.reshape(H + 1, n, width).transpose(1, 0, 2)
        ).astype(f16)

    Wq_a = tile_w(Wq, 128)
    Wk_a = tile_w(Wk, 128)
    Wv_a = tile_w(Wv, 512)
    Wg_f = np.asarray(Wg, np.float32)
    Wg_a = Wg_f.astype(f16)
    Wg_lo = (Wg_f - Wg_a.astype(np.float32)).astype(f16)
    bq_v = np.asarray(bq, np.float32)
    bk_v = np.asarray(bk, np.float32)
    bv_v = np.asarray(bv, np.float32)
    bvp_a = np.zeros((VW, NH), np.float32)
    bvp_a[0:HD, :] = bv_v.reshape(NH, HD).T
    bg_v = np.asarray(bg, np.float32).reshape(1)
    idx_all = np.arange(S, dtype=np.float32)

    in_maps = []
    for c in range(NC):
        b = c // 4
        q0 = (c % 4) * SLAB
        hT_f = hidden_states[b].T  # [H, S]
        hT_full = hT_f.astype(f16)
        hTq_f = hT_f[:, q0 : q0 + SLAB]
        hTq = hTq_f.astype(f16)
        hTq_lo = (hTq_f - hTq.astype(np.float32)).astype(f16)
        in_maps.append(
            {
                "hT": hT_full,
                "hTq": np.ascontiguousarray(hTq),
                "hTq_lo": np.ascontiguousarray(hTq_lo),
                "Wq": Wq_a,
                "Wk": Wk_a,
                "Wv": Wv_a,
                "Wg": Wg_a,
                "Wg_lo": Wg_lo,
                "bqv": bq_v,
                "bkv": bk_v,
                "bvp": bvp_a,
                "bgv": bg_v,
                "idx": idx_all,
                "idxq": np.ascontiguousarray(idx_all[q0 : q0 + SLAB]),
            }
        )
    return in_maps


def kernel(**inputs) -> np.ndarray:
    nc = _get_nc()
    in_maps = _prep_inputs(**inputs)
    res = run_bass_kernel_spmd(nc, in_maps, core_ids=list(range(NC)))
    out = np.empty((B, S, H), np.float32)
    for c in range(NC):
        b = c // 4
        q0 = (c % 4) * SLAB
        out[b, q0 : q0 + SLAB, :] = res.results[c]["out"]
    return out
